# revision 5
# baseline (speedup 1.0000x reference)
"""Hough-transform voting kernel for Trainium2 (8 NeuronCores) — dense-A matmul.

out[m, b] = (1/128) * sum_i w_i * x[m, p_i] * [bin_i == b]
          = (1/128) * sum_p A[b, p] * x[m, p],   A[b, p] = sum of w_i over
votes with (bin b, pixel p). A is built on host (pure index/weight work,
never touches x) and streamed dense fp16 through the PE:

  - bins sharded 8 ways: core i owns bins [4140*i, 4140*(i+1))
  - psum [32 maps, 512 bins] x 8 banks = 4096 bins accumulated over 128
    pixel-chunks (contraction 128 pixels per matmul); 44 tail bins in a
    second mini-pass.
  - per chunk: lhsT = x[128 pix, 32 maps] (stationary), rhs = A-slice
    [128 pix, 4096 bins] (streamed from HBM, ~1MB/chunk).

No gather/scatter on device at all; DMA-bound at ~135MB/core.
"""

import numpy as np

IM_H, IM_W = 128, 128
HT_H, HT_W = 184, 180
NB = HT_H * HT_W          # 33120 bins
NPIX = IM_H * IM_W        # 16384 pixels
NMAPS = 32
NCORES = 8
NORM = 128.0
BPC = NB // NCORES        # 4140 bins per core
MAIN = 4096               # bins in the 8-psum-bank main pass
TAIL = BPC - MAIN         # 44
NCHK = NPIX // 128        # 128 pixel chunks


def _build_program(niter=1):
    import concourse.bacc as bacc
    import concourse.mybir as mybir
    import concourse.tile as tile

    f16 = mybir.dt.float16
    f32 = mybir.dt.float32
    nc = bacc.Bacc("TRN2", target_bir_lowering=False, debug=False)
    x_d = nc.dram_tensor("xt", [128, NCHK * NMAPS], f16, kind="ExternalInput")
    am_d = nc.dram_tensor("am", [128, NCHK * MAIN], f16, kind="ExternalInput")
    at_d = nc.dram_tensor("at", [128, NCHK * TAIL], f16, kind="ExternalInput")
    ht_d = nc.dram_tensor("ht", [32, BPC], f32, kind="ExternalOutput")

    with tile.TileContext(nc) as tc:
        with (
            tc.tile_pool(name="xp", bufs=1) as xp,
            tc.tile_pool(name="apool", bufs=4) as apool,
            tc.tile_pool(name="tp", bufs=1) as tp,
            tc.tile_pool(name="op", bufs=2) as op,
            tc.tile_pool(name="ps", bufs=8, space="PSUM") as psp,
        ):
            xsb = xp.tile([128, NCHK, NMAPS], f16)
            nc.sync.dma_start(xsb[:], x_d[:])
            tsb = tp.tile([128, NCHK, TAIL], f16)
            nc.sync.dma_start(tsb[:], at_d[:])
            for it in range(niter):
                osb = op.tile([32, BPC], f32, tag="o")
                pm = [psp.tile([32, 512], f32, space="PSUM", tag="bank",
                               name=f"pm{it}_{t}")
                      for t in range(8)]
                for c in range(NCHK):
                    asb = apool.tile([128, MAIN], f16, tag="a")
                    # alternate the issuing engine so descriptor generation
                    # for chunk c+1 overlaps the transfer of chunk c
                    eng = nc.sync if c % 2 == 0 else nc.scalar
                    eng.dma_start(asb[:], am_d[:, c * MAIN:(c + 1) * MAIN])
                    for t in range(8):
                        nc.tensor.matmul(
                            pm[t][:, :],
                            lhsT=xsb[:, c, :],
                            rhs=asb[:, t * 512:(t + 1) * 512],
                            start=(c == 0), stop=(c == NCHK - 1),
                        )
                for t in range(8):
                    nc.vector.tensor_copy(
                        osb[:, t * 512:(t + 1) * 512], pm[t][:]
                    )
                pt = psp.tile([32, 512], f32, space="PSUM", tag="bank",
                              name=f"pt{it}")
                for c in range(NCHK):
                    nc.tensor.matmul(
                        pt[:, :TAIL], lhsT=xsb[:, c, :], rhs=tsb[:, c, :],
                        start=(c == 0), stop=(c == NCHK - 1),
                    )
                nc.vector.tensor_copy(osb[:, MAIN:BPC], pt[:, :TAIL])
                nc.sync.dma_start(ht_d[:], osb[:])
    nc.compile()
    return nc


def kernel(**inputs):
    from concourse import bass_utils

    x = np.asarray(inputs["x"]).astype(np.float32)
    vp = np.asarray(inputs["vote_pixel"]).astype(np.int64)
    vb = np.asarray(inputs["vote_bin"]).astype(np.int64)
    vw = np.asarray(inputs["vote_weight"]).astype(np.float32)
    b, c = x.shape[0], x.shape[1]
    xf = x.reshape(b * c, NPIX)                       # [32, 16384]

    # xtab[p, c, m] = x[m, 128c + p]
    xtab = np.ascontiguousarray(
        xf.reshape(NMAPS, NCHK, 128).transpose(2, 1, 0)
    ).astype(np.float16).reshape(128, NCHK * NMAPS)

    in_maps = []
    af32 = np.zeros(128 * NCHK * BPC, np.float32)
    for core in range(NCORES):
        lo = BPC * core
        sel = (vb >= lo) & (vb < lo + BPC)
        p, bn, w = vp[sel], vb[sel] - lo, vw[sel]
        af32[:] = 0.0
        # A layout [part=pix%128, chunk=pix//128, bin]
        flat = (p % 128) * (NCHK * BPC) + (p // 128) * BPC + bn
        np.add.at(af32, flat, w)
        a3 = af32.reshape(128, NCHK, BPC).astype(np.float16)
        am = np.ascontiguousarray(a3[:, :, :MAIN]).reshape(128, NCHK * MAIN)
        at = np.ascontiguousarray(a3[:, :, MAIN:]).reshape(128, NCHK * TAIL)
        in_maps.append({"xt": xtab, "am": am, "at": at})

    global _PROG_CACHE
    try:
        cached = _PROG_CACHE
    except NameError:
        cached = _PROG_CACHE = {}
    globals()["_LAST_IN_MAPS"] = in_maps
    if "nc" not in cached:
        cached["nc"] = _build_program()
    nc = cached["nc"]
    res = bass_utils.run_bass_kernel_spmd(nc, in_maps, core_ids=list(range(NCORES)))

    out32 = np.empty((NMAPS, NB), np.float32)
    for core in range(NCORES):
        out32[:, BPC * core:BPC * (core + 1)] = res.results[core]["ht"]
    out32 /= NORM
    return np.ascontiguousarray(out32).reshape(b, c, HT_H, HT_W)


# revision 6
# speedup vs baseline: 1.3800x; 1.3800x over previous
"""Hough-transform voting kernel for Trainium2 (8 NeuronCores) — dense-A matmul.

out[m, b] = (1/128) * sum_i w_i * x[m, p_i] * [bin_i == b]
          = (1/128) * sum_p A[b, p] * x[m, p],   A[b, p] = sum of w_i over
votes with (bin b, pixel p). A is built on host (pure index/weight work,
never touches x) and streamed dense fp16 through the PE:

  - bins sharded 8 ways: core i owns bins [4140*i, 4140*(i+1))
  - psum [32 maps, 512 bins] x 8 banks = 4096 bins accumulated over 128
    pixel-chunks (contraction 128 pixels per matmul); 44 tail bins in a
    second mini-pass.
  - per chunk: lhsT = x[128 pix, 32 maps] (stationary), rhs = A-slice
    [128 pix, 4096 bins] (streamed from HBM, ~1MB/chunk).

No gather/scatter on device at all; DMA-bound at ~135MB/core.
"""

import numpy as np

IM_H, IM_W = 128, 128
HT_H, HT_W = 184, 180
NB = HT_H * HT_W          # 33120 bins
NPIX = IM_H * IM_W        # 16384 pixels
NMAPS = 32
NCORES = 8
NORM = 128.0
BPC = NB // NCORES        # 4140 bins per core
MAIN = 4096               # bins in the 8-psum-bank main pass
TAIL = BPC - MAIN         # 44
NCHK = NPIX // 128        # 128 pixel chunks


def _build_program(niter=1):
    import concourse.bacc as bacc
    import concourse.mybir as mybir
    import concourse.tile as tile

    f16 = mybir.dt.float16
    f32 = mybir.dt.float32
    nc = bacc.Bacc("TRN2", target_bir_lowering=False, debug=False)
    x_d = nc.dram_tensor("xt", [128, NCHK * NMAPS], f16, kind="ExternalInput")
    am_d = nc.dram_tensor("am", [128, NCHK * MAIN], f16, kind="ExternalInput")
    at_d = nc.dram_tensor("at", [128, NCHK * TAIL], f16, kind="ExternalInput")
    ht_d = nc.dram_tensor("ht", [32, BPC], f32, kind="ExternalOutput")

    with tile.TileContext(nc) as tc:
        with (
            tc.tile_pool(name="xp", bufs=1) as xp,
            tc.tile_pool(name="apool", bufs=3) as apool,
            tc.tile_pool(name="tp", bufs=1) as tp,
            tc.tile_pool(name="op", bufs=2) as op,
            tc.tile_pool(name="ps", bufs=8, space="PSUM") as psp,
        ):
            xsb = xp.tile([128, NCHK, NMAPS], f16)
            nc.sync.dma_start(xsb[:], x_d[:])
            tsb = tp.tile([128, NCHK, TAIL], f16)
            nc.sync.dma_start(tsb[:], at_d[:])
            for it in range(niter):
                osb = op.tile([32, BPC], f32, tag="o")
                pm = [psp.tile([32, 512], f32, space="PSUM", tag="bank",
                               name=f"pm{it}_{t}")
                      for t in range(8)]
                for c in range(NCHK):
                    asb = apool.tile([128, MAIN], f16, tag="a")
                    nc.sync.dma_start(asb[:], am_d[:, c * MAIN:(c + 1) * MAIN])
                    for t in range(8):
                        nc.tensor.matmul(
                            pm[t][:, :],
                            lhsT=xsb[:, c, :],
                            rhs=asb[:, t * 512:(t + 1) * 512],
                            start=(c == 0), stop=(c == NCHK - 1),
                        )
                for t in range(8):
                    nc.vector.tensor_copy(
                        osb[:, t * 512:(t + 1) * 512], pm[t][:]
                    )
                pt = psp.tile([32, 512], f32, space="PSUM", tag="bank",
                              name=f"pt{it}")
                for c in range(NCHK):
                    nc.tensor.matmul(
                        pt[:, :TAIL], lhsT=xsb[:, c, :], rhs=tsb[:, c, :],
                        start=(c == 0), stop=(c == NCHK - 1),
                    )
                nc.vector.tensor_copy(osb[:, MAIN:BPC], pt[:, :TAIL])
                nc.sync.dma_start(ht_d[:], osb[:])
    nc.compile()
    return nc


def kernel(**inputs):
    from concourse import bass_utils

    x = np.asarray(inputs["x"]).astype(np.float32)
    vp = np.asarray(inputs["vote_pixel"]).astype(np.int64)
    vb = np.asarray(inputs["vote_bin"]).astype(np.int64)
    vw = np.asarray(inputs["vote_weight"]).astype(np.float32)
    b, c = x.shape[0], x.shape[1]
    xf = x.reshape(b * c, NPIX)                       # [32, 16384]

    # xtab[p, c, m] = x[m, 128c + p]
    xtab = np.ascontiguousarray(
        xf.reshape(NMAPS, NCHK, 128).transpose(2, 1, 0)
    ).astype(np.float16).reshape(128, NCHK * NMAPS)

    in_maps = []
    af32 = np.zeros(128 * NCHK * BPC, np.float32)
    for core in range(NCORES):
        lo = BPC * core
        sel = (vb >= lo) & (vb < lo + BPC)
        p, bn, w = vp[sel], vb[sel] - lo, vw[sel]
        af32[:] = 0.0
        # A layout [part=pix%128, chunk=pix//128, bin]
        flat = (p % 128) * (NCHK * BPC) + (p // 128) * BPC + bn
        np.add.at(af32, flat, w)
        a3 = af32.reshape(128, NCHK, BPC).astype(np.float16)
        am = np.ascontiguousarray(a3[:, :, :MAIN]).reshape(128, NCHK * MAIN)
        at = np.ascontiguousarray(a3[:, :, MAIN:]).reshape(128, NCHK * TAIL)
        in_maps.append({"xt": xtab, "am": am, "at": at})

    global _PROG_CACHE
    try:
        cached = _PROG_CACHE
    except NameError:
        cached = _PROG_CACHE = {}
    globals()["_LAST_IN_MAPS"] = in_maps
    if "nc" not in cached:
        cached["nc"] = _build_program()
    nc = cached["nc"]
    res = bass_utils.run_bass_kernel_spmd(nc, in_maps, core_ids=list(range(NCORES)))

    out32 = np.empty((NMAPS, NB), np.float32)
    for core in range(NCORES):
        out32[:, BPC * core:BPC * (core + 1)] = res.results[core]["ht"]
    out32 /= NORM
    return np.ascontiguousarray(out32).reshape(b, c, HT_H, HT_W)
